# revision 9
# baseline (speedup 1.0000x reference)
"""Multi-head causal attention (B=4, N=2048, C=1024, H=16) on 8 trn2 NeuronCores.

Sharding: core c -> batch b = c//2, head-group g = c%2 (8 heads each).
Each core computes qkv projection for its heads, causal attention, and a
partial output projection over its 512 attention channels; a pair-wise
ReduceScatter(add) completes the projection, each core emitting its half of
the tokens for its batch.  Host assembles the 8 [1024, 1024] results.

All matmuls run as float32r (single-pass reduced-precision fp32, ~1e-4 rel).
Attention-score layout is transposed (S^T[k_tok, q_tok]) so softmax
normalization sums arrive for free from a ones-row augmented V in the PV
matmul, and no P-transposes are needed.
"""

import os
import sys

for _p in ("/opt/trn_rl_repo",):
    if _p not in sys.path:
        sys.path.insert(0, _p)

import numpy as np

B = 4
N = 2048
C = 1024
H = 16
DK = 64
N_CORES = 8
HL = 8  # local heads per core
CL = HL * DK  # 512 local channels
PAIRS = HL // 2  # local head pairs
NT = N // 128  # 16 token tiles of 128
NQ = N // 512  # 4 query chunks of 512
KC = C // 128  # 8 embed contraction chunks

_nc_cache = None


def _build():
    import concourse.bass as bass
    import concourse.mybir as mybir
    import concourse.tile as tile
    from concourse import bacc

    f32 = mybir.dt.float32
    f32r = mybir.dt.float32r

    def _r(ap):
        return ap.bitcast(f32r)

    nc = bacc.Bacc("TRN2", target_bir_lowering=False, num_devices=N_CORES)

    x = nc.dram_tensor("x", [N, C], f32, kind="ExternalInput")
    w_q = nc.dram_tensor("w_q", [PAIRS, 128, KC, 128], f32, kind="ExternalInput")
    w_k = nc.dram_tensor("w_k", [PAIRS, 128, KC, 128], f32, kind="ExternalInput")
    w_v = nc.dram_tensor("w_v", [KC, 128, CL], f32, kind="ExternalInput")
    w_o = nc.dram_tensor("w_o", [PAIRS, 128, C], f32, kind="ExternalInput")
    b_q = nc.dram_tensor("b_q", [1, CL], f32, kind="ExternalInput")
    b_k = nc.dram_tensor("b_k", [1, CL], f32, kind="ExternalInput")
    b_v = nc.dram_tensor("b_v", [1, CL], f32, kind="ExternalInput")
    b_o2 = nc.dram_tensor("b_o2", [1, C], f32, kind="ExternalInput")
    ident_d = nc.dram_tensor("ident", [128, 128], f32, kind="ExternalInput")
    ones_d = nc.dram_tensor("ones", [1, 512], f32, kind="ExternalInput")
    out = nc.dram_tensor("out", [N // 2, C], f32, kind="ExternalOutput")

    EXP = mybir.ActivationFunctionType.Exp
    from contextlib import ExitStack

    with tile.TileContext(nc, pool_alloc_mode="queue") as tc, ExitStack() as st:
        const = st.enter_context(tc.tile_pool(name="const", bufs=1))
        ident = const.tile([128, 128], f32r)
        nc.sync.dma_start(out=ident, in_=ident_d[:, :].bitcast(f32r))
        ones = const.tile([1, 512], f32r)
        nc.sync.dma_start(out=ones, in_=ones_d[:, :].bitcast(f32r))
        bq_sb = const.tile([1, CL], f32r)
        bk_sb = const.tile([1, CL], f32r)
        bv_sb = const.tile([1, CL], f32r)
        bo_sb = const.tile([1, C], f32r)
        nc.sync.dma_start(out=bq_sb, in_=b_q[:, :].bitcast(f32r))
        nc.sync.dma_start(out=bk_sb, in_=b_k[:, :].bitcast(f32r))
        nc.sync.dma_start(out=bv_sb, in_=b_v[:, :].bitcast(f32r))
        nc.sync.dma_start(out=bo_sb, in_=b_o2[:, :].bitcast(f32r))

        qkT_pool = st.enter_context(tc.tile_pool(name="qkT", bufs=1))
        qT = [qkT_pool.tile([128, N], f32r, tag=f"qT{p}", name=f"qT{p}") for p in range(PAIRS)]
        kT = [qkT_pool.tile([128, N], f32r, tag=f"kT{p}", name=f"kT{p}") for p in range(PAIRS)]
        v_pool = st.enter_context(tc.tile_pool(name="v", bufs=1))
        vt = v_pool.tile([128, NT, HL, DK + 1], f32r, name="vt")
        ps = st.enter_context(tc.tile_pool(name="ps", bufs=1, space="PSUM"))

        xt_stack = ExitStack()
        xt_pool = xt_stack.enter_context(tc.tile_pool(name="xt", bufs=1))
        xT = [xt_pool.tile([128, N], f32r, tag=f"xt{k}", name=f"xt{k}") for k in range(KC)]

        # ---- Phase A: x^T via PE transpose ----
        with tc.tile_pool(name="xa", bufs=3) as xa_pool:
            for mt in range(NT):
                xa = xa_pool.tile([128, C], f32r)
                nc.sync.dma_start(out=xa, in_=x[mt * 128 : (mt + 1) * 128, :].bitcast(f32r))
                for kc in range(KC):
                    tp = ps.tile([128, 128], f32, tag="pj", bufs=2, name="tp")
                    nc.tensor.transpose(
                        _r(tp[:, :]), _r(xa[:, kc * 128 : (kc + 1) * 128]), _r(ident)
                    )
                    nc.vector.tensor_copy(
                        xT[kc][:, mt * 128 : (mt + 1) * 128], tp[:, :]
                    )

        # ---- Phase V: V natural [tok, chan] + ones column ----
        with tc.tile_pool(name="wv", bufs=1) as wv_pool:
            wv_sb = [wv_pool.tile([128, CL], f32r, tag=f"wv{kc}", name=f"wv{kc}") for kc in range(KC)]
            for kc in range(KC):
                nc.sync.dma_start(out=wv_sb[kc], in_=w_v[kc].bitcast(f32r))
            for mt in range(NT):
                pv = ps.tile([128, 512], f32, tag="pj", bufs=2)
                for kc in range(KC):
                    nc.tensor.matmul(
                        pv[:, :],
                        _r(xT[kc][:, mt * 128 : (mt + 1) * 128]),
                        _r(wv_sb[kc][:, :]),
                        start=(kc == 0), stop=False,
                    )
                nc.tensor.matmul(
                    pv[:, :], _r(ones[0:1, 0:128]), _r(bv_sb[0:1, :]),
                    start=False, stop=True,
                )
                nc.vector.tensor_copy(
                    vt[:, mt, :, 0:DK], pv.rearrange("p (h d) -> p h d", h=HL)
                )
                import bass_rust as _br
                ones_bcast = bass.AP(
                    tensor=ones_d, offset=0, ap=[[0, 128], [1, HL]]
                ).bitcast(f32r)
                nc.gpsimd.dma_start(out=vt[:, mt, :, DK : DK + 1], in_=ones_bcast)

        # ---- Phase B: Q^T, K^T  [chan, tok] with bias ----
        with tc.tile_pool(name="wqk", bufs=1) as wqk_pool:
            for p in range(PAIRS):
                for which, wdram, bias, dst in (
                    (0, w_q, bq_sb, qT), (1, w_k, bk_sb, kT),
                ):
                    wt = wqk_pool.tile([128, KC, 128], f32r, tag=f"w{which}")
                    nc.sync.dma_start(out=wt, in_=wdram[p].bitcast(f32r))
                    for mq in range(NQ):
                        pq = ps.tile([128, 512], f32, tag="pj", bufs=2)
                        for kc in range(KC):
                            nc.tensor.matmul(
                                pq[:, :],
                                _r(wt[:, kc, :]),
                                _r(xT[kc][:, mq * 512 : (mq + 1) * 512]),
                                start=(kc == 0), stop=False,
                            )
                        nc.tensor.matmul(
                            pq[:, :],
                            _r(bias[0:1, p * 128 : (p + 1) * 128]),
                            _r(ones[0:1, :]),
                            start=False, stop=True,
                        )
                        nc.vector.tensor_copy(
                            dst[p][:, mq * 512 : (mq + 1) * 512], pq[:, :]
                        )

        xt_stack.close()

        # ---- Phase C: attention per pair ----
        aoT_pool = st.enter_context(tc.tile_pool(name="aoT", bufs=1))
        aoT = [aoT_pool.tile([128, N], f32r, tag=f"ao{p}", name=f"aoT{p}") for p in range(PAIRS)]
        c_stack = ExitStack()
        pt_pool = c_stack.enter_context(tc.tile_pool(name="pt", bufs=2))
        rcp_pool = c_stack.enter_context(tc.tile_pool(name="rcp", bufs=4))

        for p in range(PAIRS):
            for qc in range(NQ):
                ao = [
                    ps.tile([65, 512], f32, tag=f"ao{h}", bufs=1, name=f"aops{h}") for h in range(2)
                ]
                n_kt = 4 * qc + 4
                for bb in range(n_kt // 2):
                    kts = (2 * bb, 2 * bb + 1)
                    for h in range(2):
                        rows = slice(64 * h, 64 * h + 64)
                        tpos = (64 * h, 0)
                        s_t = ps.tile([128, 1024], f32, tag=f"s{h}", bufs=1)
                        runs = []
                        for i, kt in enumerate(kts):
                            off = 128 * (kt - 4 * qc) if kt >= 4 * qc else 0
                            c0 = min(off, 256)
                            nc.tensor.matmul(
                                s_t[:, i * 512 + c0 : (i + 1) * 512],
                                _r(kT[p][rows, kt * 128 : (kt + 1) * 128]),
                                _r(qT[p][rows, qc * 512 + c0 : (qc + 1) * 512]),
                                start=True, stop=True, tile_position=tpos,
                            )
                            runs.append((i * 512 + off, (i + 1) * 512))
                        # merge adjacent exp runs
                        merged = [runs[0]]
                        for a, b in runs[1:]:
                            if merged[-1][1] == a:
                                merged[-1] = (merged[-1][0], b)
                            else:
                                merged.append((a, b))
                        pt = pt_pool.tile([128, 1024], f32r, tag=f"pt{h}")
                        for a, b in merged:
                            nc.scalar.activation(
                                pt[:, a:b], s_t[:, a:b], EXP, scale=0.125
                            )
                        for i, kt in enumerate(kts):
                            if kt >= 4 * qc:  # triangular boundary block
                                off = 128 * (kt - 4 * qc)
                                blk = slice(i * 512 + off, i * 512 + off + 128)
                                nc.gpsimd.affine_select(
                                    out=pt[:, blk], in_=pt[:, blk],
                                    compare_op=mybir.AluOpType.is_ge,
                                    fill=0.0, base=0, pattern=[[1, 128]],
                                    channel_multiplier=-1,
                                )
                        for i, kt in enumerate(kts):
                            off = 128 * (kt - 4 * qc) if kt >= 4 * qc else 0
                            nc.tensor.matmul(
                                ao[h][0:65, off:512],
                                _r(vt[:, kt, 2 * p + h, :]),
                                _r(pt[:, i * 512 + off : (i + 1) * 512]),
                                start=(kt == 0), stop=(kt == n_kt - 1),
                                skip_group_check=True,
                            )
                for h in range(2):
                    rcp = rcp_pool.tile([1, 512], f32)
                    nc.vector.reciprocal(rcp[:, :], ao[h][64:65, :])
                    rcpb = rcp_pool.tile([64, 512], f32, tag="rcpb", bufs=2, name="rcpb")
                    nc.gpsimd.partition_broadcast(rcpb[:, :], rcp[0:1, :])
                    nc.vector.tensor_tensor(
                        aoT[p][64 * h : 64 * h + 64, qc * 512 : (qc + 1) * 512],
                        ao[h][0:64, :],
                        rcpb[:, :],
                        mybir.AluOpType.mult,
                    )

        c_stack.close()

        # ---- Phase D: output projection (partial) + bias/2 ----
        dram = st.enter_context(tc.tile_pool(name="dram", bufs=1, space="DRAM"))
        rs_in = dram.tile([N, C], f32)
        rs_out = dram.tile([N // 2, C], f32)
        with tc.tile_pool(name="wo", bufs=1) as wo_pool, tc.tile_pool(
            name="ob", bufs=3
        ) as ob_pool:
            wo_sb = [wo_pool.tile([128, C], f32r, tag=f"wo{cc}", name=f"wo{cc}") for cc in range(PAIRS)]
            for cc in range(PAIRS):
                nc.sync.dma_start(out=wo_sb[cc], in_=w_o[cc].bitcast(f32r))
            for mt in range(NT):
                for nn in range(2):
                    pj = ps.tile([128, 512], f32, tag="pj", bufs=2)
                    for cc in range(PAIRS):
                        nc.tensor.matmul(
                            pj[:, :],
                            _r(aoT[cc][:, mt * 128 : (mt + 1) * 128]),
                            _r(wo_sb[cc][:, nn * 512 : (nn + 1) * 512]),
                            start=(cc == 0), stop=False,
                        )
                    nc.tensor.matmul(
                        pj[:, :],
                        _r(ones[0:1, 0:128]),
                        _r(bo_sb[0:1, nn * 512 : (nn + 1) * 512]),
                        start=False, stop=True,
                    )
                    ob = ob_pool.tile([128, 512], f32)
                    nc.vector.tensor_copy(ob[:, :], pj[:, :])
                    nc.sync.dma_start(
                        out=rs_in[mt * 128 : (mt + 1) * 128, nn * 512 : (nn + 1) * 512],
                        in_=ob[:, :],
                    )

        nc.gpsimd.collective_compute(
            "ReduceScatter",
            mybir.AluOpType.add,
            replica_groups=[[0, 1], [2, 3], [4, 5], [6, 7]],
            ins=[rs_in.opt()],
            outs=[rs_out.opt()],
        )
        nc.sync.dma_start(out=out[:, :], in_=rs_out[:, :])

    nc.compile()
    return nc


def _get_nc():
    global _nc_cache
    if _nc_cache is None:
        _nc_cache = _build()
    return _nc_cache


def kernel(x, W_qkv, b_qkv, W_o, b_o):
    from concourse.bass_utils import run_bass_kernel_spmd

    x = np.asarray(x, dtype=np.float32)
    W_qkv = np.asarray(W_qkv, dtype=np.float32)
    b_qkv = np.asarray(b_qkv, dtype=np.float32)
    W_o = np.asarray(W_o, dtype=np.float32)
    b_o = np.asarray(b_o, dtype=np.float32)

    in_maps = []
    for c in range(N_CORES):
        b, g = divmod(c, 2)
        cs = slice(CL * g, CL * (g + 1))
        W_q_c = W_qkv[:, 0:C][:, cs]
        W_k_c = W_qkv[:, C : 2 * C][:, cs]
        W_v_c = W_qkv[:, 2 * C : 3 * C][:, cs]
        in_maps.append(
            {
                "x": np.ascontiguousarray(x[b]),
                "w_q": np.ascontiguousarray(
                    W_q_c.reshape(KC, 128, PAIRS, 128).transpose(2, 1, 0, 3)
                ),
                "w_k": np.ascontiguousarray(
                    W_k_c.reshape(KC, 128, PAIRS, 128).transpose(2, 1, 0, 3)
                ),
                "w_v": np.ascontiguousarray(W_v_c.reshape(KC, 128, CL)),
                "w_o": np.ascontiguousarray(W_o[cs, :].reshape(PAIRS, 128, C)),
                "b_q": np.ascontiguousarray(b_qkv[0:C][cs][None, :]),
                "b_k": np.ascontiguousarray(b_qkv[C : 2 * C][cs][None, :]),
                "b_v": np.ascontiguousarray(b_qkv[2 * C : 3 * C][cs][None, :]),
                "b_o2": np.ascontiguousarray((0.5 * b_o)[None, :]),
                "ident": np.eye(128, dtype=np.float32),
                "ones": np.ones((1, 512), dtype=np.float32),
            }
        )

    nc = _get_nc()
    trace = bool(int(os.environ.get("BASS_KERNEL_TRACE", "0")))
    tmpdir = os.environ.get("BASS_KERNEL_TRACE_DIR") or None
    res = run_bass_kernel_spmd(
        nc, in_maps, list(range(N_CORES)), trace=trace, tmpdir=tmpdir
    )
    kernel.last_result = res

    full = np.empty((B, N, C), dtype=np.float32)
    for c in range(N_CORES):
        b, g = divmod(c, 2)
        full[b, g * (N // 2) : (g + 1) * (N // 2), :] = res.results[c]["out"]
    return full


kernel.last_result = None


# revision 18
# speedup vs baseline: 1.1760x; 1.1760x over previous
"""Multi-head causal attention (B=4, N=2048, C=1024, H=16) on 8 trn2 NeuronCores.

Sharding: core c -> batch b = c//2, head-group g = c%2 (8 heads each).
Each core computes qkv projection for its heads, causal attention, and a
partial output projection over its 512 attention channels; a pair-wise
ReduceScatter(add) completes the projection, each core emitting its half of
the tokens for its batch.  Host assembles the 8 [1024, 1024] results.

All matmuls run as float32r (single-pass reduced-precision fp32, ~1e-4 rel).
Attention-score layout is transposed (S^T[k_tok, q_tok]) so softmax
normalization sums arrive for free from a ones-row augmented V in the PV
matmul, and no P-transposes are needed.
"""

import os
import sys

for _p in ("/opt/trn_rl_repo",):
    if _p not in sys.path:
        sys.path.insert(0, _p)

import numpy as np

B = 4
N = 2048
C = 1024
H = 16
DK = 64
N_CORES = 8
HL = 8  # local heads per core
CL = HL * DK  # 512 local channels
PAIRS = HL // 2  # local head pairs
NT = N // 128  # 16 token tiles of 128
NQ = N // 512  # 4 query chunks of 512
KC = C // 128  # 8 embed contraction chunks

_nc_cache = None


def _build():
    import concourse.bass as bass
    import concourse.mybir as mybir
    import concourse.tile as tile
    from concourse import bacc
    from contextlib import ExitStack

    f32 = mybir.dt.float32
    f32r = mybir.dt.float32r

    def _r(ap):
        return ap.bitcast(f32r)

    nc = bacc.Bacc("TRN2", target_bir_lowering=False, num_devices=N_CORES)

    x = nc.dram_tensor("x", [N, C], f32, kind="ExternalInput")
    w_q = nc.dram_tensor("w_q", [PAIRS, 128, KC, 128], f32, kind="ExternalInput")
    w_k = nc.dram_tensor("w_k", [PAIRS, 128, KC, 128], f32, kind="ExternalInput")
    w_v = nc.dram_tensor("w_v", [KC, 128, CL], f32, kind="ExternalInput")
    w_o = nc.dram_tensor("w_o", [PAIRS, 128, C], f32, kind="ExternalInput")
    b_q = nc.dram_tensor("b_q", [1, CL], f32, kind="ExternalInput")
    b_k = nc.dram_tensor("b_k", [1, CL], f32, kind="ExternalInput")
    b_v = nc.dram_tensor("b_v", [1, CL], f32, kind="ExternalInput")
    b_o2 = nc.dram_tensor("b_o2", [1, C], f32, kind="ExternalInput")
    ident_d = nc.dram_tensor("ident", [128, 128], f32, kind="ExternalInput")
    tri_d = nc.dram_tensor("tri", [128, 128], f32, kind="ExternalInput")
    ones_d = nc.dram_tensor("ones", [1, 512], f32, kind="ExternalInput")
    out = nc.dram_tensor("out", [N // 2, C], f32, kind="ExternalOutput")

    EXP = mybir.ActivationFunctionType.Exp

    with tile.TileContext(nc, pool_alloc_mode="queue") as tc, ExitStack() as st:
        # ---------- permanent pools ----------
        const = st.enter_context(tc.tile_pool(name="const", bufs=1))
        ident = const.tile([128, 128], f32r)
        nc.sync.dma_start(out=ident, in_=ident_d[:, :].bitcast(f32r))
        ones = const.tile([1, 512], f32r)
        nc.sync.dma_start(out=ones, in_=ones_d[:, :].bitcast(f32r))
        tri_sb = const.tile([128, 128], f32r)
        nc.sync.dma_start(out=tri_sb, in_=tri_d[:, :].bitcast(f32r))
        bq_sb = const.tile([1, CL], f32r)
        bk_sb = const.tile([1, CL], f32r)
        bv_sb = const.tile([1, CL], f32r)
        bo_sb = const.tile([1, C], f32r)
        nc.sync.dma_start(out=bq_sb, in_=b_q[:, :].bitcast(f32r))
        nc.sync.dma_start(out=bk_sb, in_=b_k[:, :].bitcast(f32r))
        nc.sync.dma_start(out=bv_sb, in_=b_v[:, :].bitcast(f32r))
        nc.sync.dma_start(out=bo_sb, in_=b_o2[:, :].bitcast(f32r))

        v_pool = st.enter_context(tc.tile_pool(name="v", bufs=1))
        vt = v_pool.tile([128, NT, HL, DK + 1], f32r, name="vt")
        qkT_pool = st.enter_context(tc.tile_pool(name="qkT", bufs=1))
        qT = [
            qkT_pool.tile([128, N], f32r, tag=f"qT{p}", name=f"qT{p}")
            for p in range(PAIRS)
        ]
        kT = [
            qkT_pool.tile([128, N], f32r, tag=f"kT{p}", name=f"kT{p}")
            for p in range(PAIRS)
        ]
        ps = st.enter_context(tc.tile_pool(name="ps", bufs=1, space="PSUM"))
        dram = st.enter_context(tc.tile_pool(name="dram", bufs=1, space="DRAM"))
        rs_in = dram.tile([N, C], f32)
        rs_out = dram.tile([N // 2, C], f32)

        # ---------- phase A/V/B transient pools (LIFO) ----------
        ab_stack = ExitStack()
        xt_pool = ab_stack.enter_context(tc.tile_pool(name="xt", bufs=1))
        xT = [
            xt_pool.tile([128, N], f32r, tag=f"xt{k}", name=f"xt{k}")
            for k in range(KC)
        ]
        wv_stack = ExitStack()
        wv_pool = wv_stack.enter_context(tc.tile_pool(name="wv", bufs=1))
        wv_sb = [
            wv_pool.tile([128, CL], f32r, tag=f"wv{kc}", name=f"wv{kc}")
            for kc in range(KC)
        ]
        for kc in range(KC):
            nc.sync.dma_start(out=wv_sb[kc], in_=w_v[kc].bitcast(f32r))

        # ---- Phase A: x^T via PE transpose ----
        with tc.tile_pool(name="xa", bufs=3) as xa_pool:
            for mt in range(NT):
                xa = xa_pool.tile([128, C], f32r)
                nc.sync.dma_start(
                    out=xa, in_=x[mt * 128 : (mt + 1) * 128, :].bitcast(f32r)
                )
                for kc in range(KC):
                    tp = ps.tile([128, 128], f32, tag="pj", bufs=2, name="tp")
                    nc.tensor.transpose(
                        _r(tp[:, :]), _r(xa[:, kc * 128 : (kc + 1) * 128]), _r(ident)
                    )
                    nc.vector.tensor_copy(
                        xT[kc][:, mt * 128 : (mt + 1) * 128], tp[:, :]
                    )

        # ---- Phase V: V natural [tok, chan] + ones column ----
        for mt in range(NT):
            pv = ps.tile([128, 512], f32, tag="pj", bufs=2)
            for kc in range(KC):
                nc.tensor.matmul(
                    pv[:, :],
                    _r(xT[kc][:, mt * 128 : (mt + 1) * 128]),
                    _r(wv_sb[kc][:, :]),
                    start=(kc == 0), stop=False,
                )
            nc.tensor.matmul(
                pv[:, :], _r(ones[0:1, 0:128]), _r(bv_sb[0:1, :]),
                start=False, stop=True,
            )
            nc.vector.tensor_copy(
                vt[:, mt, :, 0:DK], pv.rearrange("p (h d) -> p h d", h=HL)
            )
            ones_bcast = bass.AP(
                tensor=ones_d, offset=0, ap=[[0, 128], [1, HL]]
            ).bitcast(f32r)
            nc.sync.dma_start(out=vt[:, mt, :, DK : DK + 1], in_=ones_bcast)
        wv_stack.close()

        # ---- Phase B: Q^T, K^T  [chan, tok] with bias ----
        wqk_pool = ab_stack.enter_context(tc.tile_pool(name="wqk", bufs=2))
        for p in range(PAIRS):
            for which, wdram, bias, dst in (
                (0, w_q, bq_sb, qT), (1, w_k, bk_sb, kT),
            ):
                wt = wqk_pool.tile(
                    [128, KC, 128], f32r, tag=f"w{which}", name=f"w{which}_{p}"
                )
                nc.sync.dma_start(out=wt, in_=wdram[p].bitcast(f32r))
                for mq in range(NQ):
                    pq = ps.tile([128, 512], f32, tag="pj", bufs=2)
                    for kc in range(KC):
                        nc.tensor.matmul(
                            pq[:, :],
                            _r(wt[:, kc, :]),
                            _r(xT[kc][:, mq * 512 : (mq + 1) * 512]),
                            start=(kc == 0), stop=False,
                        )
                    nc.tensor.matmul(
                        pq[:, :],
                        _r(bias[0:1, p * 128 : (p + 1) * 128]),
                        _r(ones[0:1, :]),
                        start=False, stop=True,
                    )
                    nc.vector.tensor_copy(
                        dst[p][:, mq * 512 : (mq + 1) * 512], pq[:, :]
                    )
        ab_stack.close()

        # ---- Phase C: attention per pair ----
        aoT_pool = st.enter_context(tc.tile_pool(name="aoT", bufs=1))
        aoT = [
            aoT_pool.tile([128, N], f32r, tag=f"ao{p}", name=f"aoT{p}")
            for p in range(PAIRS)
        ]
        c_stack = ExitStack()
        pt_pool = c_stack.enter_context(tc.tile_pool(name="pt", bufs=2))
        rcp_pool = c_stack.enter_context(tc.tile_pool(name="rcp", bufs=3))

        for p in range(PAIRS):
            for qc in range(NQ):
                ao = [
                    ps.tile([65, 512], f32, tag=f"ao{h}", bufs=1, name=f"aops{h}")
                    for h in range(2)
                ]
                n_kt = 4 * qc + 4
                for bb in range(n_kt // 2):
                    kts = (2 * bb, 2 * bb + 1)
                    for h in range(2):
                        rows = slice(64 * h, 64 * h + 64)
                        tpos = (64 * h, 0)
                        s_t = ps.tile([128, 1024], f32, tag=f"s{h}", bufs=1)
                        runs = []
                        for i, kt in enumerate(kts):
                            off = 128 * (kt - 4 * qc) if kt >= 4 * qc else 0
                            c0 = min(off, 256)
                            nc.tensor.matmul(
                                s_t[:, i * 512 + c0 : (i + 1) * 512],
                                _r(kT[p][rows, kt * 128 : (kt + 1) * 128]),
                                _r(qT[p][rows, qc * 512 + c0 : (qc + 1) * 512]),
                                start=True, stop=True, tile_position=tpos,
                            )
                            runs.append((i * 512 + off, (i + 1) * 512))
                        merged = [runs[0]]
                        for a, b in runs[1:]:
                            if merged[-1][1] == a:
                                merged[-1] = (merged[-1][0], b)
                            else:
                                merged.append((a, b))
                        pt = pt_pool.tile([128, 1024], f32r, tag=f"pt{h}")
                        for a, b in merged:
                            nc.scalar.activation(
                                pt[:, a:b], s_t[:, a:b], EXP, scale=0.125
                            )
                        for i, kt in enumerate(kts):
                            if kt >= 4 * qc:  # triangular boundary block
                                off = 128 * (kt - 4 * qc)
                                blk = slice(i * 512 + off, i * 512 + off + 128)
                                nc.vector.tensor_tensor(
                                    pt[:, blk], pt[:, blk], tri_sb[:, :],
                                    mybir.AluOpType.mult,
                                )
                        for i, kt in enumerate(kts):
                            off = 128 * (kt - 4 * qc) if kt >= 4 * qc else 0
                            nc.tensor.matmul(
                                ao[h][0:65, off:512],
                                _r(vt[:, kt, 2 * p + h, :]),
                                _r(pt[:, i * 512 + off : (i + 1) * 512]),
                                start=(kt == 0), stop=(kt == n_kt - 1),
                                skip_group_check=True,
                            )
                for h in range(2):
                    tmp = rcp_pool.tile([65, 512], f32, tag="tmp", bufs=3, name="tmp")
                    nc.vector.tensor_copy(tmp[0:64, :], ao[h][0:64, :])
                    rs_row = rcp_pool.tile([1, 512], f32, tag="rsr", bufs=3, name="rsr")
                    nc.vector.tensor_copy(rs_row[0:1, :], ao[h][64:65, :])
                    rcp = rcp_pool.tile([1, 512], f32, tag="rcp", bufs=3, name="rcp")
                    nc.vector.reciprocal_approx_fast(rcp[:, :], rs_row[0:1, :])
                    rcp_d = dram.tile([1, 512], f32, tag="rcpd", bufs=3, name="rcpd")
                    nc.sync.dma_start(out=rcp_d[:, :], in_=rcp[:, :])
                    rcpb = rcp_pool.tile([64, 512], f32, tag="rcpb", bufs=3, name="rcpb")
                    nc.sync.dma_start(
                        out=rcpb[:, :], in_=rcp_d[0:1, :].partition_broadcast(64)
                    )
                    nc.vector.tensor_tensor(
                        aoT[p][64 * h : 64 * h + 64, qc * 512 : (qc + 1) * 512],
                        tmp[0:64, :],
                        rcpb[:, :],
                        mybir.AluOpType.mult,
                    )
        c_stack.close()

        # ---- Phase D: output projection (partial) + bias/2, chunked RS ----
        with tc.tile_pool(name="wo", bufs=1) as wo_pool, tc.tile_pool(
            name="ob", bufs=3
        ) as ob_pool:
            wo_sb = [
                wo_pool.tile([128, C], f32r, tag=f"wo{cc}", name=f"wo{cc}")
                for cc in range(PAIRS)
            ]
            for cc in range(PAIRS):
                nc.sync.dma_start(out=wo_sb[cc], in_=w_o[cc].bitcast(f32r))
            for mt in range(NT):
                for nn in range(2):
                    pj = ps.tile([128, 512], f32, tag="pj", bufs=2)
                    for cc in range(PAIRS):
                        nc.tensor.matmul(
                            pj[:, :],
                            _r(aoT[cc][:, mt * 128 : (mt + 1) * 128]),
                            _r(wo_sb[cc][:, nn * 512 : (nn + 1) * 512]),
                            start=(cc == 0), stop=False,
                        )
                    nc.tensor.matmul(
                        pj[:, :],
                        _r(ones[0:1, 0:128]),
                        _r(bo_sb[0:1, nn * 512 : (nn + 1) * 512]),
                        start=False, stop=True,
                    )
                    ob = ob_pool.tile([128, 512], f32)
                    nc.vector.tensor_copy(ob[:, :], pj[:, :])
                    nc.sync.dma_start(
                        out=rs_in[
                            mt * 128 : (mt + 1) * 128, nn * 512 : (nn + 1) * 512
                        ],
                        in_=ob[:, :],
                    )
                if mt % 4 == 3:  # rows [512*(mt//4) : +512) complete -> RS chunk
                    ch = mt // 4
                    nc.gpsimd.collective_compute(
                        "ReduceScatter",
                        mybir.AluOpType.add,
                        replica_groups=[[0, 1], [2, 3], [4, 5], [6, 7]],
                        ins=[rs_in[ch * 512 : (ch + 1) * 512, :].opt()],
                        outs=[rs_out[ch * 256 : (ch + 1) * 256, :].opt()],
                    )
                    nc.sync.dma_start(
                        out=out[ch * 256 : (ch + 1) * 256, :],
                        in_=rs_out[ch * 256 : (ch + 1) * 256, :],
                    )

    nc.compile()
    return nc


def _get_nc():
    global _nc_cache
    if _nc_cache is None:
        _nc_cache = _build()
    return _nc_cache


def kernel(x, W_qkv, b_qkv, W_o, b_o):
    from concourse.bass_utils import run_bass_kernel_spmd

    x = np.asarray(x, dtype=np.float32)
    W_qkv = np.asarray(W_qkv, dtype=np.float32)
    b_qkv = np.asarray(b_qkv, dtype=np.float32)
    W_o = np.asarray(W_o, dtype=np.float32)
    b_o = np.asarray(b_o, dtype=np.float32)

    in_maps = []
    for c in range(N_CORES):
        b, g = divmod(c, 2)
        cs = slice(CL * g, CL * (g + 1))
        W_q_c = W_qkv[:, 0:C][:, cs]
        W_k_c = W_qkv[:, C : 2 * C][:, cs]
        W_v_c = W_qkv[:, 2 * C : 3 * C][:, cs]
        in_maps.append(
            {
                "x": np.ascontiguousarray(x[b]),
                "w_q": np.ascontiguousarray(
                    W_q_c.reshape(KC, 128, PAIRS, 128).transpose(2, 1, 0, 3)
                ),
                "w_k": np.ascontiguousarray(
                    W_k_c.reshape(KC, 128, PAIRS, 128).transpose(2, 1, 0, 3)
                ),
                "w_v": np.ascontiguousarray(W_v_c.reshape(KC, 128, CL)),
                "w_o": np.ascontiguousarray(W_o[cs, :].reshape(PAIRS, 128, C)),
                "b_q": np.ascontiguousarray(b_qkv[0:C][cs][None, :]),
                "b_k": np.ascontiguousarray(b_qkv[C : 2 * C][cs][None, :]),
                "b_v": np.ascontiguousarray(b_qkv[2 * C : 3 * C][cs][None, :]),
                "b_o2": np.ascontiguousarray((0.5 * b_o)[None, :]),
                "ident": np.eye(128, dtype=np.float32),
                "tri": np.triu(np.ones((128, 128), dtype=np.float32)),
                "ones": np.ones((1, 512), dtype=np.float32),
            }
        )

    nc = _get_nc()
    trace = bool(int(os.environ.get("BASS_KERNEL_TRACE", "0")))
    tmpdir = os.environ.get("BASS_KERNEL_TRACE_DIR") or None
    res = run_bass_kernel_spmd(
        nc, in_maps, list(range(N_CORES)), trace=trace, tmpdir=tmpdir
    )
    kernel.last_result = res

    full = np.empty((B, N, C), dtype=np.float32)
    for c in range(N_CORES):
        b, rank = divmod(c, 2)
        o = res.results[c]["out"]
        for ch in range(4):
            t0 = 512 * ch + 256 * rank
            full[b, t0 : t0 + 256, :] = o[256 * ch : 256 * (ch + 1), :]
    return full


kernel.last_result = None
